# revision 9
# baseline (speedup 1.0000x reference)
"""Trainium2 Bass kernel for DinkNet-style GNN message passing (8 NeuronCores).

Pipeline per core (SPMD, identical instruction stream, per-core data):
  phase 1: h = x @ W for this core's node shard (both feature sets, bf16,
           pair-interleaved rows [node, 256] = [h1 | h2])
  phase 2: AllGather h shards -> full h table in local DRAM
  phase 3: for each owned dest tile (128 nodes): gather source rows via
           dma_gather (4 source blocks, int16 indices), build a one-hot
           selection matrix M[e,d] = w_e * (dest_local(e) == d) on DVE,
           segment-sum via PE matmuls accumulating in PSUM [feat, dest],
           epilogue: relu(agg+b), relu(-agg-b) on ACT, matvec with
           sum(lin_W, axis=1) on PE, z = r1 - a*r2 + sum(lin_b) on DVE.

Host side: partition edges by dest owner, group by (dest_tile, src_block),
pad each group to 128-edge tiles with a schedule shared by all 8 cores
(max over cores), so the single compiled program fits every core's data.
"""

import sys

sys.path.insert(0, "/opt/trn_rl_repo")

import numpy as np
import ml_dtypes

N_NODES = 100000
N_EDGES = 800000
N_IN = 256
N_H = 128
NC_CORES = 8
SH = N_NODES // NC_CORES          # 12500 nodes per core
ND_TILES = (SH + 127) // 128      # 98 dest tiles per core
SHP = ND_TILES * 128              # 12544 padded shard rows
BLK = 2 * SHP                     # 25088 source rows per gather block
NBLK = 4
CHUNK = 32                        # tiles (of 128 edges) per dma_gather call
BF16 = ml_dtypes.bfloat16


def _wrap_idx(stream: np.ndarray) -> np.ndarray:
    """int16 stream [L] -> wrapped [128, L//16]: idx j at (j%16, j//16),
    replicated across the 8 groups of 16 partitions."""
    L = stream.shape[0]
    assert L % 16 == 0
    w16 = stream.reshape(L // 16, 16).T  # [16, L//16]
    return np.tile(w16, (8, 1)).astype(np.int16)


def prepare(edge_index, edge_weight):
    """Partition + pad edges. Returns (TK, per-core arrays)."""
    row = np.asarray(edge_index[0], dtype=np.int64)
    col = np.asarray(edge_index[1], dtype=np.int64)
    w = np.asarray(edge_weight, dtype=np.float32)

    src_row = (col // SH) * SHP + (col % SH)      # padded global h row
    blk = src_row // BLK
    idx_local = (src_row % BLK).astype(np.int64)

    owner = row // SH
    dest_local = row - owner * SH                 # 0..SH-1
    k_tile = dest_local // 128

    # counts[c, k, b]
    counts = np.zeros((NC_CORES, ND_TILES, NBLK), dtype=np.int64)
    np.add.at(counts, (owner, k_tile, blk), 1)
    TK = np.ceil(counts.max(axis=0) / 128.0).astype(np.int64)  # [ND_TILES, NBLK]
    TK[:, 0] = np.maximum(TK[:, 0], 1)            # ensure psum groups get written

    Tb = TK.sum(axis=0)                           # tiles per block
    T_total = int(TK.sum())

    cores = []
    order = np.lexsort((idx_local, blk, k_tile, owner))
    row_s = dest_local[order]
    idx_s, w_s = idx_local[order], w[order]
    # start offset of each (c,k,b) group in the sorted arrays
    flat_counts = counts.reshape(-1)
    flat_starts = np.concatenate(([0], np.cumsum(flat_counts)[:-1])).reshape(
        NC_CORES, ND_TILES, NBLK
    )

    for c in range(NC_CORES):
        idx_streams = [np.zeros(int(Tb[b]) * 128, dtype=np.int16) for b in range(NBLK)]
        dloc = np.full((T_total, 128), 200.0, dtype=np.float32)
        wts = np.zeros((T_total, 128), dtype=np.float32)
        bpos = [0] * NBLK
        t = 0
        for k in range(ND_TILES):
            for b in range(NBLK):
                n_t = int(TK[k, b])
                if n_t == 0:
                    continue
                s0 = int(flat_starts[c, k, b])
                n_e = int(counts[c, k, b])
                cap = n_t * 128
                assert n_e <= cap
                seg_idx = idx_s[s0 : s0 + n_e].astype(np.int16)
                seg_d = (row_s[s0 : s0 + n_e] - k * 128).astype(np.float32)
                seg_w = w_s[s0 : s0 + n_e]
                pad_idx = seg_idx[-1] if n_e > 0 else np.int16(0)
                buf_idx = np.full(cap, pad_idx, dtype=np.int16)
                buf_idx[:n_e] = seg_idx
                p0 = bpos[b] * 128
                idx_streams[b][p0 : p0 + cap] = buf_idx
                dloc[t : t + n_t].reshape(-1)[:n_e] = seg_d
                wts[t : t + n_t].reshape(-1)[:n_e] = seg_w
                bpos[b] += n_t
                t += n_t
        assert t == T_total
        cores.append(
            dict(
                idx=[_wrap_idx(s) for s in idx_streams],
                dloc=np.ascontiguousarray(dloc.T),   # [128, T_total]
                wts=np.ascontiguousarray(wts.T),     # [128, T_total]
            )
        )
    return TK, Tb, T_total, cores


def build_program(TK, Tb, T_total, alpha, bsum, stage="full", repeat=1):
    import concourse.bacc as bacc
    import concourse.tile as tile
    from concourse import mybir
    from contextlib import ExitStack

    f32, bf16, i16 = mybir.dt.float32, mybir.dt.bfloat16, mybir.dt.int16
    AO = mybir.AluOpType

    nc = bacc.Bacc("TRN2", target_bir_lowering=False, debug=False,
                   num_devices=NC_CORES, num_swdge_queues=4)
    dbg_outs = {}
    if stage == "p1":
        dbg_outs["hdbg"] = nc.dram_tensor("hdbg", [SHP, 2 * N_H], bf16,
                                          kind="ExternalOutput")
    elif stage == "ag":
        dbg_outs["hfull_dbg"] = nc.dram_tensor(
            "hfull_dbg", [NC_CORES * SHP, 2 * N_H], bf16, kind="ExternalOutput")
    elif stage == "g1":
        for b in range(NBLK):
            dbg_outs[f"gdbg{b}"] = nc.dram_tensor(
                f"gdbg{b}", [128, int(Tb[b]), 2 * N_H], bf16,
                kind="ExternalOutput")

    xT_in = nc.dram_tensor("xT_in", [N_IN, 2 * SHP], bf16, kind="ExternalInput")
    W_in = nc.dram_tensor("w_in", [N_IN, N_H], bf16, kind="ExternalInput")
    iota_in = nc.dram_tensor("iota_in", [128, 128], f32, kind="ExternalInput")
    bias_in = nc.dram_tensor("bias_in", [128, 2], f32, kind="ExternalInput")
    wsum_in = nc.dram_tensor("wsum_in", [128, 1], bf16, kind="ExternalInput")
    dloc_in = nc.dram_tensor("dloc_in", [128, T_total], f32, kind="ExternalInput")
    wts_in = nc.dram_tensor("wts_in", [128, T_total], f32, kind="ExternalInput")
    idx_ins = [
        nc.dram_tensor(f"idx{b}_in", [128, int(Tb[b]) * 8], i16, kind="ExternalInput")
        for b in range(NBLK)
    ]
    z_out = nc.dram_tensor("z_out", [2, SHP], f32, kind="ExternalOutput")

    with tile.TileContext(nc) as tc, ExitStack() as ctx:
        dram = ctx.enter_context(tc.tile_pool(name="dram", bufs=1, space="DRAM"))
        h_shard = dram.tile([SHP, 2 * N_H], bf16)

        const = ctx.enter_context(tc.tile_pool(name="const", bufs=1))
        W_sb = const.tile([128, 2, N_H], bf16)
        nc.sync.dma_start(W_sb[:], W_in.ap().rearrange("(a p) h -> p a h", a=2))
        iota_sb = const.tile([128, 128], f32)
        nc.sync.dma_start(iota_sb[:], iota_in.ap())
        bias_sb = const.tile([128, 2], f32)
        nc.sync.dma_start(bias_sb[:], bias_in.ap())
        wsum_sb = const.tile([128, 1], bf16)
        nc.sync.dma_start(wsum_sb[:], wsum_in.ap())
        dloc_sb = const.tile([128, T_total], f32)
        nc.sync.dma_start(dloc_sb[:], dloc_in.ap())
        wts_sb = const.tile([128, T_total], f32)
        nc.sync.dma_start(wts_sb[:], wts_in.ap())

        for _rep in range(repeat):
            h_full = dram.tile([NC_CORES * SHP, 2 * N_H], bf16,
                               addr_space="Shared", tag=f"hf{_rep % 2}")
            # ---- phase 1: h = x @ W (both gcns) ----
            with tc.tile_pool(name="p1x", bufs=4) as p1x, \
                 tc.tile_pool(name="p1ps", bufs=2, space="PSUM") as p1ps, \
                 tc.tile_pool(name="p1h", bufs=3) as p1h:
                xg = xT_in.ap().rearrange("p (g n) -> p g n", g=2)
                for j in range(ND_TILES):
                    sl = slice(j * 128, (j + 1) * 128)
                    xa = p1x.tile([128, 2, 128], bf16, tag="xa")
                    nc.sync.dma_start(xa[:], xg[0:128, :, sl])
                    xb = p1x.tile([128, 2, 128], bf16, tag="xb")
                    nc.sync.dma_start(xb[:], xg[128:256, :, sl])
                    ph = p1ps.tile([128, 256], f32)
                    for g in range(2):
                        nc.tensor.matmul(ph[:, g * 128:(g + 1) * 128],
                                         lhsT=xa[:, g, :], rhs=W_sb[:, 0, :],
                                         start=True, stop=False)
                        nc.tensor.matmul(ph[:, g * 128:(g + 1) * 128],
                                         lhsT=xb[:, g, :], rhs=W_sb[:, 1, :],
                                         start=False, stop=True)
                    hs = p1h.tile([128, 256], bf16)
                    nc.vector.tensor_copy(hs[:], ph[:])
                    nc.sync.dma_start(h_shard[sl, :], hs[:])

            if stage == "p1":
                continue
            if stage == "p1t":
                nc.sync.dma_start(z_out.ap()[0:2, 0:128], dloc_sb[:, 0:2])
                continue

            if stage == "monly":
                # p1 + M-build only: no AG, no gather, no matmul
                with tc.tile_pool(name="mm", bufs=4) as mp, \
                     tc.tile_pool(name="acc", bufs=2) as accp:
                    for t_global in range(T_total):
                        M = mp.tile([128, 128], bf16)
                        nc.vector.tensor_scalar(
                            M[:], iota_sb[:],
                            dloc_sb[:, t_global:t_global + 1],
                            wts_sb[:, t_global:t_global + 1],
                            AO.is_equal, AO.mult)
                        if t_global % 64 == 63:
                            acc = accp.tile([128, 1], bf16, tag="acc")
                            nc.vector.tensor_reduce(
                                acc[:], M[:, 0:2], mybir.AxisListType.X, AO.max)
                nc.sync.dma_start(z_out.ap()[0:2, 0:128], dloc_sb[:, 0:2])
                continue

            # ---- phase 2: AllGather ----
            nc.gpsimd.collective_compute(
                "AllGather", AO.bypass,
                replica_groups=[list(range(NC_CORES))],
                ins=[h_shard[:]], outs=[h_full[:]],
            )

            if stage == "ag":
                continue
            if stage == "agt":
                nc.sync.dma_start(z_out.ap()[0:2, 0:128], dloc_sb[:, 0:2])
                continue

            if stage == "g1":
                with tc.tile_pool(name="msg", bufs=2) as msgp, \
                     tc.tile_pool(name="idx", bufs=4) as idxp:
                    for b in range(NBLK):
                        tb = 0
                        while tb < int(Tb[b]):
                            nchunk = min(CHUNK, int(Tb[b]) - tb)
                            it = idxp.tile([128, nchunk * 8], i16, tag=f"idx{b}")
                            nc.sync.dma_start(
                                it[:], idx_ins[b].ap()[:, tb * 8:(tb + nchunk) * 8])
                            mt = msgp.tile([128, nchunk, 2 * N_H], bf16,
                                           tag=f"msg{b}")
                            nc.gpsimd.dma_gather(
                                mt[:], h_full[b * BLK:(b + 1) * BLK, :], it[:],
                                nchunk * 128, nchunk * 128, 2 * N_H,
                                single_packet=False, queue_num=b % 4)
                            nc.sync.dma_start(
                                dbg_outs[f"gdbg{b}"].ap()[:, tb:tb + nchunk, :],
                                mt[:])
                            tb += nchunk
                continue

            if stage == "g2":
                with tc.tile_pool(name="msg", bufs=2) as msgp, \
                     tc.tile_pool(name="idx", bufs=4) as idxp, \
                     tc.tile_pool(name="acc", bufs=2) as accp:
                    for b in range(NBLK):
                        tb = 0
                        while tb < int(Tb[b]):
                            nchunk = min(CHUNK, int(Tb[b]) - tb)
                            it = idxp.tile([128, nchunk * 8], i16, tag=f"idx{b}")
                            nc.sync.dma_start(
                                it[:], idx_ins[b].ap()[:, tb * 8:(tb + nchunk) * 8])
                            mt = msgp.tile([128, nchunk, 2 * N_H], bf16,
                                           tag=f"msg{b}")
                            nc.gpsimd.dma_gather(
                                mt[:], h_full[b * BLK:(b + 1) * BLK, :], it[:],
                                nchunk * 128, nchunk * 128, 2 * N_H,
                                single_packet=False, queue_num=b % 4)
                            acc = accp.tile([128, 1], f32, tag=f"acc{b}")
                            nc.vector.tensor_reduce(
                                acc[:], mt[:, 0, 0:2], mybir.AxisListType.X,
                                AO.max)
                            tb += nchunk
                continue

            # ---- phase 3: gather + segment-sum + epilogue ----
            with tc.tile_pool(name="msg", bufs=2) as msgp, \
                 tc.tile_pool(name="idx", bufs=4) as idxp, \
                 tc.tile_pool(name="mm", bufs=4) as mp, \
                 tc.tile_pool(name="sps", bufs=2, space="PSUM") as sps, \
                 tc.tile_pool(name="hs2", bufs=3) as hs2p, \
                 tc.tile_pool(name="zps", bufs=2, space="PSUM") as zps, \
                 tc.tile_pool(name="ztmp", bufs=4) as ztp:
                tile_cnt = [0] * NBLK
                cur = [None] * NBLK
                t_global = 0
                for k in range(ND_TILES):
                    ps1 = sps.tile([128, 128], f32, tag="ps1")
                    ps2 = sps.tile([128, 128], f32, tag="ps2")
                    tiles_k = [(b, i) for b in range(NBLK)
                               for i in range(int(TK[k, b]))]
                    for ti, (b, _) in enumerate(tiles_k):
                        tb = tile_cnt[b]
                        if cur[b] is None or tb - cur[b][1] >= CHUNK:
                            nchunk = min(CHUNK, int(Tb[b]) - tb)
                            mt = msgp.tile([128, nchunk, 2 * N_H], bf16,
                                           tag=f"msg{b}")
                            if stage != "nogather":
                                it = idxp.tile([128, nchunk * 8], i16,
                                               tag=f"idx{b}")
                                nc.sync.dma_start(
                                    it[:],
                                    idx_ins[b].ap()[:, tb * 8:(tb + nchunk) * 8])
                                nc.gpsimd.dma_gather(
                                    mt[:], h_full[b * BLK:(b + 1) * BLK, :],
                                    it[:], nchunk * 128, nchunk * 128, 2 * N_H,
                                    single_packet=False, queue_num=b % 4)
                            cur[b] = (mt, tb)
                        mt, base = cur[b]
                        slot = tb - base
                        M = mp.tile([128, 128], bf16)
                        nc.vector.tensor_scalar(
                            M[:], iota_sb[:],
                            dloc_sb[:, t_global:t_global + 1],
                            wts_sb[:, t_global:t_global + 1],
                            AO.is_equal, AO.mult)
                        first = ti == 0
                        last = ti == len(tiles_k) - 1
                        nc.tensor.matmul(ps1[:], lhsT=mt[:, slot, 0:N_H],
                                         rhs=M[:], start=first, stop=last)
                        nc.tensor.matmul(ps2[:], lhsT=mt[:, slot, N_H:2 * N_H],
                                         rhs=M[:], start=first, stop=last)
                        tile_cnt[b] += 1
                        t_global += 1

                    hs = hs2p.tile([128, 512], bf16)
                    ACT = mybir.ActivationFunctionType.Relu
                    nc.scalar.activation(hs[:, 0:128], ps1[:], ACT,
                                         bias=bias_sb[:, 0:1], scale=1.0)
                    nc.scalar.activation(hs[:, 128:256], ps1[:], ACT,
                                         bias=bias_sb[:, 1:2], scale=-1.0)
                    nc.scalar.activation(hs[:, 256:384], ps2[:], ACT,
                                         bias=bias_sb[:, 0:1], scale=1.0)
                    nc.scalar.activation(hs[:, 384:512], ps2[:], ACT,
                                         bias=bias_sb[:, 1:2], scale=-1.0)
                    zp = zps.tile([1, 512], f32)
                    nc.tensor.matmul(zp[:], lhsT=wsum_sb[:], rhs=hs[:],
                                     start=True, stop=True)
                    wk = SH - k * 128 if (k + 1) * 128 > SH else 128
                    c0 = k * 128
                    tz1 = ztp.tile([1, 128], f32, tag="tz1")
                    nc.vector.tensor_scalar(tz1[:, :wk], zp[0:1, 128:128 + wk],
                                            -alpha, bsum, AO.mult, AO.add)
                    zf1 = ztp.tile([1, 128], f32, tag="zf1")
                    nc.vector.tensor_tensor(zf1[:, :wk], zp[0:1, 0:wk],
                                            tz1[:, :wk], op=AO.add)
                    nc.sync.dma_start(z_out.ap()[0:1, c0:c0 + wk], zf1[:, :wk])
                    tz2 = ztp.tile([1, 128], f32, tag="tz2")
                    nc.vector.tensor_scalar(tz2[:, :wk], zp[0:1, 384:384 + wk],
                                            -alpha, bsum, AO.mult, AO.add)
                    zf2 = ztp.tile([1, 128], f32, tag="zf2")
                    nc.vector.tensor_tensor(zf2[:, :wk], zp[0:1, 256:256 + wk],
                                            tz2[:, :wk], op=AO.add)
                    nc.sync.dma_start(z_out.ap()[1:2, c0:c0 + wk], zf2[:, :wk])

        if stage == "p1":
            nc.sync.dma_start(dbg_outs["hdbg"].ap(), h_shard[:])
        elif stage == "ag":
            nc.sync.dma_start(dbg_outs["hfull_dbg"].ap(), h_full[:])

    nc.compile()
    return nc


def make_in_maps(x_1, x_2, W_gcn, gcn_bias, lin_W, cores, T_total):
    x1 = np.asarray(x_1)[0]
    x2 = np.asarray(x_2)[0]
    W_bf = np.asarray(W_gcn, dtype=np.float32).astype(BF16)
    iota = np.tile(np.arange(128, dtype=np.float32), (128, 1))
    bias2 = np.stack([np.asarray(gcn_bias, np.float32),
                      -np.asarray(gcn_bias, np.float32)], axis=1)
    wsum = np.asarray(lin_W, np.float32).sum(axis=1, keepdims=True).astype(BF16)
    in_maps = []
    for c in range(NC_CORES):
        sl = slice(c * SH, (c + 1) * SH)
        xT = np.zeros((N_IN, 2 * SHP), dtype=BF16)
        xT[:, :SH] = x1[sl].T.astype(BF16)
        xT[:, SHP:SHP + SH] = x2[sl].T.astype(BF16)
        m = {
            "xT_in": xT,
            "w_in": W_bf,
            "iota_in": iota,
            "bias_in": np.ascontiguousarray(bias2),
            "wsum_in": wsum,
            "dloc_in": cores[c]["dloc"],
            "wts_in": cores[c]["wts"],
        }
        for b in range(NBLK):
            m[f"idx{b}_in"] = cores[c]["idx"][b]
        in_maps.append(m)
    return in_maps


def kernel(x_1, x_2, edge_index, edge_weight, W_gcn, gcn_bias, prelu_a,
           lin_W, lin_b):
    from concourse.bass_utils import run_bass_kernel_spmd

    TK, Tb, T_total, cores = prepare(edge_index, edge_weight)
    alpha = float(np.asarray(prelu_a).reshape(-1)[0])
    bsum = float(np.asarray(lin_b, dtype=np.float32).sum())
    nc = build_program(TK, Tb, T_total, alpha, bsum)
    in_maps = make_in_maps(x_1, x_2, W_gcn, gcn_bias, lin_W, cores, T_total)
    res = run_bass_kernel_spmd(nc, in_maps, core_ids=list(range(NC_CORES)))
    z1 = np.concatenate([res.results[c]["z_out"][0, :SH] for c in range(NC_CORES)])
    z2 = np.concatenate([res.results[c]["z_out"][1, :SH] for c in range(NC_CORES)])
    return np.concatenate([z1, z2]).astype(np.float32)



# revision 10
# speedup vs baseline: 1.2602x; 1.2602x over previous
"""Trainium2 Bass kernel for DinkNet-style GNN message passing (8 NeuronCores).

Pipeline per core (SPMD, identical instruction stream, per-core data):
  phase 1: h = x @ W for this core's node shard (both feature sets, bf16,
           pair-interleaved rows [node, 256] = [h1 | h2])
  phase 2: AllGather h shards -> full h table in local DRAM
  phase 3: for each owned dest tile (128 nodes): gather source rows via
           dma_gather (4 source blocks, int16 indices), build a one-hot
           selection matrix M[e,d] = w_e * (dest_local(e) == d) on DVE,
           segment-sum via PE matmuls accumulating in PSUM [feat, dest],
           epilogue: relu(agg+b), relu(-agg-b) on ACT, matvec with
           sum(lin_W, axis=1) on PE, z = r1 - a*r2 + sum(lin_b) on DVE.

Host side: partition edges by dest owner, group by (dest_tile, src_block),
pad each group to 128-edge tiles with a schedule shared by all 8 cores
(max over cores), so the single compiled program fits every core's data.
"""

import sys

sys.path.insert(0, "/opt/trn_rl_repo")

import numpy as np
import ml_dtypes

N_NODES = 100000
N_EDGES = 800000
N_IN = 256
N_H = 128
NC_CORES = 8
SH = N_NODES // NC_CORES          # 12500 nodes per core
ND_TILES = (SH + 127) // 128      # 98 dest tiles per core
SHP = ND_TILES * 128              # 12544 padded shard rows
BLK = 2 * SHP                     # 25088 source rows per gather block
NBLK = 4
CHUNK = 32                        # tiles (of 128 edges) per dma_gather call
BF16 = ml_dtypes.bfloat16


def _wrap_idx(stream: np.ndarray) -> np.ndarray:
    """int16 stream [L] -> wrapped [128, L//16]: idx j at (j%16, j//16),
    replicated across the 8 groups of 16 partitions."""
    L = stream.shape[0]
    assert L % 16 == 0
    w16 = stream.reshape(L // 16, 16).T  # [16, L//16]
    return np.tile(w16, (8, 1)).astype(np.int16)


def prepare(edge_index, edge_weight):
    """Partition + pad edges. Returns (TK, per-core arrays)."""
    row = np.asarray(edge_index[0], dtype=np.int64)
    col = np.asarray(edge_index[1], dtype=np.int64)
    w = np.asarray(edge_weight, dtype=np.float32)

    src_row = (col // SH) * SHP + (col % SH)      # padded global h row
    blk = src_row // BLK
    idx_local = (src_row % BLK).astype(np.int64)

    owner = row // SH
    dest_local = row - owner * SH                 # 0..SH-1
    k_tile = dest_local // 128

    # counts[c, k, b]
    counts = np.zeros((NC_CORES, ND_TILES, NBLK), dtype=np.int64)
    np.add.at(counts, (owner, k_tile, blk), 1)
    TK = np.ceil(counts.max(axis=0) / 128.0).astype(np.int64)  # [ND_TILES, NBLK]
    TK[:, 0] = np.maximum(TK[:, 0], 1)            # ensure psum groups get written

    Tb = TK.sum(axis=0)                           # tiles per block
    T_total = int(TK.sum())

    cores = []
    order = np.lexsort((idx_local, blk, k_tile, owner))
    row_s = dest_local[order]
    idx_s, w_s = idx_local[order], w[order]
    # start offset of each (c,k,b) group in the sorted arrays
    flat_counts = counts.reshape(-1)
    flat_starts = np.concatenate(([0], np.cumsum(flat_counts)[:-1])).reshape(
        NC_CORES, ND_TILES, NBLK
    )

    for c in range(NC_CORES):
        idx_streams = [np.zeros(int(Tb[b]) * 128, dtype=np.int16) for b in range(NBLK)]
        dloc = np.full((T_total, 128), 200.0, dtype=np.float32)
        wts = np.zeros((T_total, 128), dtype=np.float32)
        bpos = [0] * NBLK
        t = 0
        for k in range(ND_TILES):
            for b in range(NBLK):
                n_t = int(TK[k, b])
                if n_t == 0:
                    continue
                s0 = int(flat_starts[c, k, b])
                n_e = int(counts[c, k, b])
                cap = n_t * 128
                assert n_e <= cap
                seg_idx = idx_s[s0 : s0 + n_e].astype(np.int16)
                seg_d = (row_s[s0 : s0 + n_e] - k * 128).astype(np.float32)
                seg_w = w_s[s0 : s0 + n_e]
                pad_idx = seg_idx[-1] if n_e > 0 else np.int16(0)
                buf_idx = np.full(cap, pad_idx, dtype=np.int16)
                buf_idx[:n_e] = seg_idx
                p0 = bpos[b] * 128
                idx_streams[b][p0 : p0 + cap] = buf_idx
                dloc[t : t + n_t].reshape(-1)[:n_e] = seg_d
                wts[t : t + n_t].reshape(-1)[:n_e] = seg_w
                bpos[b] += n_t
                t += n_t
        assert t == T_total
        cores.append(
            dict(
                idx=[_wrap_idx(s) for s in idx_streams],
                dloc=np.ascontiguousarray(dloc.T),   # [128, T_total]
                wts=np.ascontiguousarray(wts.T),     # [128, T_total]
            )
        )
    return TK, Tb, T_total, cores


def build_program(TK, Tb, T_total, alpha, bsum, stage="full", repeat=1):
    import concourse.bacc as bacc
    import concourse.tile as tile
    from concourse import mybir
    from contextlib import ExitStack

    f32, bf16, i16 = mybir.dt.float32, mybir.dt.bfloat16, mybir.dt.int16
    AO = mybir.AluOpType

    nc = bacc.Bacc("TRN2", target_bir_lowering=False, debug=False,
                   num_devices=NC_CORES, num_swdge_queues=4)
    dbg_outs = {}
    if stage == "p1":
        dbg_outs["hdbg"] = nc.dram_tensor("hdbg", [SHP, 2 * N_H], bf16,
                                          kind="ExternalOutput")
    elif stage == "ag":
        dbg_outs["hfull_dbg"] = nc.dram_tensor(
            "hfull_dbg", [NC_CORES * SHP, 2 * N_H], bf16, kind="ExternalOutput")
    elif stage == "g1":
        for b in range(NBLK):
            dbg_outs[f"gdbg{b}"] = nc.dram_tensor(
                f"gdbg{b}", [128, int(Tb[b]), 2 * N_H], bf16,
                kind="ExternalOutput")

    xT_in = nc.dram_tensor("xT_in", [N_IN, 2 * SHP], bf16, kind="ExternalInput")
    W_in = nc.dram_tensor("w_in", [N_IN, N_H], bf16, kind="ExternalInput")
    iota_in = nc.dram_tensor("iota_in", [128, 128], f32, kind="ExternalInput")
    bias_in = nc.dram_tensor("bias_in", [128, 2], f32, kind="ExternalInput")
    wsum_in = nc.dram_tensor("wsum_in", [128, 1], bf16, kind="ExternalInput")
    dloc_in = nc.dram_tensor("dloc_in", [128, T_total], f32, kind="ExternalInput")
    wts_in = nc.dram_tensor("wts_in", [128, T_total], f32, kind="ExternalInput")
    idx_ins = [
        nc.dram_tensor(f"idx{b}_in", [128, int(Tb[b]) * 8], i16, kind="ExternalInput")
        for b in range(NBLK)
    ]
    z_out = nc.dram_tensor("z_out", [2, SHP], f32, kind="ExternalOutput")

    with tile.TileContext(nc) as tc, ExitStack() as ctx:
        dram = ctx.enter_context(tc.tile_pool(name="dram", bufs=1, space="DRAM"))
        h_shard = dram.tile([SHP, 2 * N_H], bf16)

        const = ctx.enter_context(tc.tile_pool(name="const", bufs=1))
        W_sb = const.tile([128, 2, N_H], bf16)
        nc.sync.dma_start(W_sb[:], W_in.ap().rearrange("(a p) h -> p a h", a=2))
        iota_sb = const.tile([128, 128], f32)
        nc.sync.dma_start(iota_sb[:], iota_in.ap())
        bias_sb = const.tile([128, 2], f32)
        nc.sync.dma_start(bias_sb[:], bias_in.ap())
        wsum_sb = const.tile([128, 1], bf16)
        nc.sync.dma_start(wsum_sb[:], wsum_in.ap())
        dloc_sb = const.tile([128, T_total], f32)
        nc.sync.dma_start(dloc_sb[:], dloc_in.ap())
        wts_sb = const.tile([128, T_total], f32)
        nc.sync.dma_start(wts_sb[:], wts_in.ap())

        for _rep in range(repeat):
            h_full = dram.tile([NC_CORES * SHP, 2 * N_H], bf16,
                               addr_space="Shared", tag=f"hf{_rep % 2}")
            # ---- phase 1: h = x @ W (both gcns) ----
            with tc.tile_pool(name="p1x", bufs=4) as p1x, \
                 tc.tile_pool(name="p1ps", bufs=2, space="PSUM") as p1ps, \
                 tc.tile_pool(name="p1h", bufs=3) as p1h:
                xg = xT_in.ap().rearrange("p (g n) -> p g n", g=2)
                for j in range(ND_TILES):
                    sl = slice(j * 128, (j + 1) * 128)
                    xa = p1x.tile([128, 2, 128], bf16, tag="xa")
                    nc.sync.dma_start(xa[:], xg[0:128, :, sl])
                    xb = p1x.tile([128, 2, 128], bf16, tag="xb")
                    nc.sync.dma_start(xb[:], xg[128:256, :, sl])
                    ph = p1ps.tile([128, 256], f32)
                    for g in range(2):
                        nc.tensor.matmul(ph[:, g * 128:(g + 1) * 128],
                                         lhsT=xa[:, g, :], rhs=W_sb[:, 0, :],
                                         start=True, stop=False)
                        nc.tensor.matmul(ph[:, g * 128:(g + 1) * 128],
                                         lhsT=xb[:, g, :], rhs=W_sb[:, 1, :],
                                         start=False, stop=True)
                    hs = p1h.tile([128, 256], bf16)
                    nc.vector.tensor_copy(hs[:], ph[:])
                    nc.sync.dma_start(h_shard[sl, :], hs[:])

            if stage == "p1":
                continue
            if stage == "p1t":
                nc.sync.dma_start(z_out.ap()[0:2, 0:128], dloc_sb[:, 0:2])
                continue

            if stage == "monly":
                # p1 + M-build only: no AG, no gather, no matmul
                with tc.tile_pool(name="mm", bufs=4) as mp, \
                     tc.tile_pool(name="acc", bufs=2) as accp:
                    for t_global in range(T_total):
                        M = mp.tile([128, 128], bf16)
                        nc.vector.tensor_scalar(
                            M[:], iota_sb[:],
                            dloc_sb[:, t_global:t_global + 1],
                            wts_sb[:, t_global:t_global + 1],
                            AO.is_equal, AO.mult)
                        if t_global % 64 == 63:
                            acc = accp.tile([128, 1], bf16, tag="acc")
                            nc.vector.tensor_reduce(
                                acc[:], M[:, 0:2], mybir.AxisListType.X, AO.max)
                nc.sync.dma_start(z_out.ap()[0:2, 0:128], dloc_sb[:, 0:2])
                continue

            # ---- phase 2: AllGather ----
            nc.gpsimd.collective_compute(
                "AllGather", AO.bypass,
                replica_groups=[list(range(NC_CORES))],
                ins=[h_shard[:]], outs=[h_full[:]],
            )

            if stage == "ag":
                continue
            if stage == "agt":
                nc.sync.dma_start(z_out.ap()[0:2, 0:128], dloc_sb[:, 0:2])
                continue

            if stage == "g1":
                with tc.tile_pool(name="msg", bufs=2) as msgp, \
                     tc.tile_pool(name="idx", bufs=4) as idxp:
                    for b in range(NBLK):
                        tb = 0
                        while tb < int(Tb[b]):
                            nchunk = min(CHUNK, int(Tb[b]) - tb)
                            it = idxp.tile([128, nchunk * 8], i16, tag=f"idx{b}")
                            nc.sync.dma_start(
                                it[:], idx_ins[b].ap()[:, tb * 8:(tb + nchunk) * 8])
                            mt = msgp.tile([128, nchunk, 2 * N_H], bf16,
                                           tag=f"msg{b}")
                            nc.gpsimd.dma_gather(
                                mt[:], h_full[b * BLK:(b + 1) * BLK, :], it[:],
                                nchunk * 128, nchunk * 128, 2 * N_H,
                                single_packet=False, queue_num=b % 4)
                            nc.sync.dma_start(
                                dbg_outs[f"gdbg{b}"].ap()[:, tb:tb + nchunk, :],
                                mt[:])
                            tb += nchunk
                continue

            if stage == "g3":
                # half-width gather (256B rows): desc-count identical to g2
                with tc.tile_pool(name="msg", bufs=2) as msgp, \
                     tc.tile_pool(name="idx", bufs=4) as idxp, \
                     tc.tile_pool(name="acc", bufs=2) as accp:
                    for b in range(NBLK):
                        tb = 0
                        while tb < int(Tb[b]):
                            nchunk = min(CHUNK, int(Tb[b]) - tb)
                            it = idxp.tile([128, nchunk * 8], i16, tag=f"idx{b}")
                            nc.sync.dma_start(
                                it[:], idx_ins[b].ap()[:, tb * 8:(tb + nchunk) * 8])
                            mt = msgp.tile([128, nchunk, N_H], bf16,
                                           tag=f"msg{b}")
                            nc.gpsimd.dma_gather(
                                mt[:], h_full[b * BLK:(b + 1) * BLK, 0:N_H],
                                it[:], nchunk * 128, nchunk * 128, N_H,
                                elem_step=2 * N_H,
                                single_packet=False, queue_num=b % 4)
                            acc = accp.tile([128, 1], f32, tag=f"acc{b}")
                            nc.vector.tensor_reduce(
                                acc[:], mt[:, 0, 0:2], mybir.AxisListType.X,
                                AO.max)
                            tb += nchunk
                continue

            if stage == "g2":
                with tc.tile_pool(name="msg", bufs=2) as msgp, \
                     tc.tile_pool(name="idx", bufs=4) as idxp, \
                     tc.tile_pool(name="acc", bufs=2) as accp:
                    for b in range(NBLK):
                        tb = 0
                        while tb < int(Tb[b]):
                            nchunk = min(CHUNK, int(Tb[b]) - tb)
                            it = idxp.tile([128, nchunk * 8], i16, tag=f"idx{b}")
                            nc.sync.dma_start(
                                it[:], idx_ins[b].ap()[:, tb * 8:(tb + nchunk) * 8])
                            mt = msgp.tile([128, nchunk, 2 * N_H], bf16,
                                           tag=f"msg{b}")
                            nc.gpsimd.dma_gather(
                                mt[:], h_full[b * BLK:(b + 1) * BLK, :], it[:],
                                nchunk * 128, nchunk * 128, 2 * N_H,
                                single_packet=False, queue_num=b % 4)
                            acc = accp.tile([128, 1], f32, tag=f"acc{b}")
                            nc.vector.tensor_reduce(
                                acc[:], mt[:, 0, 0:2], mybir.AxisListType.X,
                                AO.max)
                            tb += nchunk
                continue

            # ---- phase 3: gather + segment-sum + epilogue ----
            with tc.tile_pool(name="msg", bufs=2) as msgp, \
                 tc.tile_pool(name="idx", bufs=4) as idxp, \
                 tc.tile_pool(name="mm", bufs=4) as mp, \
                 tc.tile_pool(name="sps", bufs=2, space="PSUM") as sps, \
                 tc.tile_pool(name="hs2", bufs=3) as hs2p, \
                 tc.tile_pool(name="zps", bufs=2, space="PSUM") as zps, \
                 tc.tile_pool(name="ztmp", bufs=4) as ztp:
                tile_cnt = [0] * NBLK
                cur = [None] * NBLK
                t_global = 0
                for k in range(ND_TILES):
                    ps1 = sps.tile([128, 128], f32, tag="ps1")
                    ps2 = sps.tile([128, 128], f32, tag="ps2")
                    tiles_k = [(b, i) for b in range(NBLK)
                               for i in range(int(TK[k, b]))]
                    for ti, (b, _) in enumerate(tiles_k):
                        tb = tile_cnt[b]
                        if cur[b] is None or tb - cur[b][1] >= CHUNK:
                            nchunk = min(CHUNK, int(Tb[b]) - tb)
                            mt = msgp.tile([128, nchunk, 2 * N_H], bf16,
                                           tag=f"msg{b}")
                            if stage != "nogather":
                                it = idxp.tile([128, nchunk * 8], i16,
                                               tag=f"idx{b}")
                                nc.sync.dma_start(
                                    it[:],
                                    idx_ins[b].ap()[:, tb * 8:(tb + nchunk) * 8])
                                nc.gpsimd.dma_gather(
                                    mt[:], h_full[b * BLK:(b + 1) * BLK, :],
                                    it[:], nchunk * 128, nchunk * 128, 2 * N_H,
                                    single_packet=False, queue_num=b % 4)
                            cur[b] = (mt, tb)
                        mt, base = cur[b]
                        slot = tb - base
                        M = mp.tile([128, 128], bf16)
                        nc.vector.tensor_scalar(
                            M[:], iota_sb[:],
                            dloc_sb[:, t_global:t_global + 1],
                            wts_sb[:, t_global:t_global + 1],
                            AO.is_equal, AO.mult)
                        first = ti == 0
                        last = ti == len(tiles_k) - 1
                        nc.tensor.matmul(ps1[:], lhsT=mt[:, slot, 0:N_H],
                                         rhs=M[:], start=first, stop=last)
                        nc.tensor.matmul(ps2[:], lhsT=mt[:, slot, N_H:2 * N_H],
                                         rhs=M[:], start=first, stop=last)
                        tile_cnt[b] += 1
                        t_global += 1

                    hs = hs2p.tile([128, 512], bf16)
                    ACT = mybir.ActivationFunctionType.Relu
                    nc.scalar.activation(hs[:, 0:128], ps1[:], ACT,
                                         bias=bias_sb[:, 0:1], scale=1.0)
                    nc.scalar.activation(hs[:, 128:256], ps1[:], ACT,
                                         bias=bias_sb[:, 1:2], scale=-1.0)
                    nc.scalar.activation(hs[:, 256:384], ps2[:], ACT,
                                         bias=bias_sb[:, 0:1], scale=1.0)
                    nc.scalar.activation(hs[:, 384:512], ps2[:], ACT,
                                         bias=bias_sb[:, 1:2], scale=-1.0)
                    zp = zps.tile([1, 512], f32)
                    nc.tensor.matmul(zp[:], lhsT=wsum_sb[:], rhs=hs[:],
                                     start=True, stop=True)
                    wk = SH - k * 128 if (k + 1) * 128 > SH else 128
                    c0 = k * 128
                    tz1 = ztp.tile([1, 128], f32, tag="tz1")
                    nc.vector.tensor_scalar(tz1[:, :wk], zp[0:1, 128:128 + wk],
                                            -alpha, bsum, AO.mult, AO.add)
                    zf1 = ztp.tile([1, 128], f32, tag="zf1")
                    nc.vector.tensor_tensor(zf1[:, :wk], zp[0:1, 0:wk],
                                            tz1[:, :wk], op=AO.add)
                    nc.sync.dma_start(z_out.ap()[0:1, c0:c0 + wk], zf1[:, :wk])
                    tz2 = ztp.tile([1, 128], f32, tag="tz2")
                    nc.vector.tensor_scalar(tz2[:, :wk], zp[0:1, 384:384 + wk],
                                            -alpha, bsum, AO.mult, AO.add)
                    zf2 = ztp.tile([1, 128], f32, tag="zf2")
                    nc.vector.tensor_tensor(zf2[:, :wk], zp[0:1, 256:256 + wk],
                                            tz2[:, :wk], op=AO.add)
                    nc.sync.dma_start(z_out.ap()[1:2, c0:c0 + wk], zf2[:, :wk])

        if stage == "p1":
            nc.sync.dma_start(dbg_outs["hdbg"].ap(), h_shard[:])
        elif stage == "ag":
            nc.sync.dma_start(dbg_outs["hfull_dbg"].ap(), h_full[:])

    nc.compile()
    return nc


def make_in_maps(x_1, x_2, W_gcn, gcn_bias, lin_W, cores, T_total):
    x1 = np.asarray(x_1)[0]
    x2 = np.asarray(x_2)[0]
    W_bf = np.asarray(W_gcn, dtype=np.float32).astype(BF16)
    iota = np.tile(np.arange(128, dtype=np.float32), (128, 1))
    bias2 = np.stack([np.asarray(gcn_bias, np.float32),
                      -np.asarray(gcn_bias, np.float32)], axis=1)
    wsum = np.asarray(lin_W, np.float32).sum(axis=1, keepdims=True).astype(BF16)
    in_maps = []
    for c in range(NC_CORES):
        sl = slice(c * SH, (c + 1) * SH)
        xT = np.zeros((N_IN, 2 * SHP), dtype=BF16)
        xT[:, :SH] = x1[sl].T.astype(BF16)
        xT[:, SHP:SHP + SH] = x2[sl].T.astype(BF16)
        m = {
            "xT_in": xT,
            "w_in": W_bf,
            "iota_in": iota,
            "bias_in": np.ascontiguousarray(bias2),
            "wsum_in": wsum,
            "dloc_in": cores[c]["dloc"],
            "wts_in": cores[c]["wts"],
        }
        for b in range(NBLK):
            m[f"idx{b}_in"] = cores[c]["idx"][b]
        in_maps.append(m)
    return in_maps


def kernel(x_1, x_2, edge_index, edge_weight, W_gcn, gcn_bias, prelu_a,
           lin_W, lin_b):
    from concourse.bass_utils import run_bass_kernel_spmd

    TK, Tb, T_total, cores = prepare(edge_index, edge_weight)
    alpha = float(np.asarray(prelu_a).reshape(-1)[0])
    bsum = float(np.asarray(lin_b, dtype=np.float32).sum())
    nc = build_program(TK, Tb, T_total, alpha, bsum)
    in_maps = make_in_maps(x_1, x_2, W_gcn, gcn_bias, lin_W, cores, T_total)
    res = run_bass_kernel_spmd(nc, in_maps, core_ids=list(range(NC_CORES)))
    z1 = np.concatenate([res.results[c]["z_out"][0, :SH] for c in range(NC_CORES)])
    z2 = np.concatenate([res.results[c]["z_out"][1, :SH] for c in range(NC_CORES)])
    return np.concatenate([z1, z2]).astype(np.float32)



# revision 11
# speedup vs baseline: 2.5818x; 2.0487x over previous
"""Trainium2 Bass kernel for DinkNet-style GNN message passing (8 NeuronCores).

Pipeline per core (SPMD, identical instruction stream, per-core data):
  phase 1: h = x @ W for this core's node shard (both feature sets, bf16,
           pair-interleaved rows [node, 256] = [h1 | h2])
  phase 2: AllGather h shards -> full h table in local DRAM
  phase 3: for each owned dest tile (128 nodes): gather source rows via
           dma_gather (4 source blocks, int16 indices), build a one-hot
           selection matrix M[e,d] = w_e * (dest_local(e) == d) on DVE,
           segment-sum via PE matmuls accumulating in PSUM [feat, dest],
           epilogue: relu(agg+b), relu(-agg-b) on ACT, matvec with
           sum(lin_W, axis=1) on PE, z = r1 - a*r2 + sum(lin_b) on DVE.

Host side: partition edges by dest owner, group by (dest_tile, src_block),
pad each group to 128-edge tiles with a schedule shared by all 8 cores
(max over cores), so the single compiled program fits every core's data.
"""

import sys

sys.path.insert(0, "/opt/trn_rl_repo")

import numpy as np
import ml_dtypes

N_NODES = 100000
N_EDGES = 800000
N_IN = 256
N_H = 128
NC_CORES = 8
SH = N_NODES // NC_CORES          # 12500 nodes per core
ND_TILES = (SH + 127) // 128      # 98 dest tiles per core
SHP = ND_TILES * 128              # 12544 padded shard rows
BLK = 2 * SHP                     # 25088 source rows per gather block
NBLK = 4
CHUNK = 32                        # tiles (of 128 edges) per dma_gather call
BF16 = ml_dtypes.bfloat16


def _wrap_idx(stream: np.ndarray) -> np.ndarray:
    """int16 stream [L] -> wrapped [128, L//16]: idx j at (j%16, j//16),
    replicated across the 8 groups of 16 partitions."""
    L = stream.shape[0]
    assert L % 16 == 0
    w16 = stream.reshape(L // 16, 16).T  # [16, L//16]
    return np.tile(w16, (8, 1)).astype(np.int16)


def prepare(edge_index, edge_weight):
    """Partition + pad edges. Returns (TK, per-core arrays)."""
    row = np.asarray(edge_index[0], dtype=np.int64)
    col = np.asarray(edge_index[1], dtype=np.int64)
    w = np.asarray(edge_weight, dtype=np.float32)

    src_row = (col // SH) * SHP + (col % SH)      # padded global h row
    blk = src_row // BLK
    idx_local = (src_row % BLK).astype(np.int64)

    owner = row // SH
    dest_local = row - owner * SH                 # 0..SH-1
    k_tile = dest_local // 128

    # counts[c, k, b]
    counts = np.zeros((NC_CORES, ND_TILES, NBLK), dtype=np.int64)
    np.add.at(counts, (owner, k_tile, blk), 1)
    TK = np.ceil(counts.max(axis=0) / 128.0).astype(np.int64)  # [ND_TILES, NBLK]
    TK[:, 0] = np.maximum(TK[:, 0], 1)            # ensure psum groups get written

    Tb = TK.sum(axis=0)                           # tiles per block
    T_total = int(TK.sum())

    cores = []
    order = np.lexsort((idx_local, blk, k_tile, owner))
    row_s = dest_local[order]
    idx_s, w_s = idx_local[order], w[order]
    # start offset of each (c,k,b) group in the sorted arrays
    flat_counts = counts.reshape(-1)
    flat_starts = np.concatenate(([0], np.cumsum(flat_counts)[:-1])).reshape(
        NC_CORES, ND_TILES, NBLK
    )

    for c in range(NC_CORES):
        idx_streams = [np.zeros(int(Tb[b]) * 128, dtype=np.int16) for b in range(NBLK)]
        dloc = np.full((T_total, 128), 200.0, dtype=np.float32)
        wts = np.zeros((T_total, 128), dtype=np.float32)
        bpos = [0] * NBLK
        t = 0
        for k in range(ND_TILES):
            for b in range(NBLK):
                n_t = int(TK[k, b])
                if n_t == 0:
                    continue
                s0 = int(flat_starts[c, k, b])
                n_e = int(counts[c, k, b])
                cap = n_t * 128
                assert n_e <= cap
                seg_idx = idx_s[s0 : s0 + n_e].astype(np.int16)
                seg_d = (row_s[s0 : s0 + n_e] - k * 128).astype(np.float32)
                seg_w = w_s[s0 : s0 + n_e]
                pad_idx = seg_idx[-1] if n_e > 0 else np.int16(0)
                buf_idx = np.full(cap, pad_idx, dtype=np.int16)
                buf_idx[:n_e] = seg_idx
                p0 = bpos[b] * 128
                idx_streams[b][p0 : p0 + cap] = buf_idx
                dloc[t : t + n_t].reshape(-1)[:n_e] = seg_d
                wts[t : t + n_t].reshape(-1)[:n_e] = seg_w
                bpos[b] += n_t
                t += n_t
        assert t == T_total
        cores.append(
            dict(
                idx=[_wrap_idx(s) for s in idx_streams],
                dloc=np.ascontiguousarray(dloc.T),   # [128, T_total]
                wts=np.ascontiguousarray(wts.T),     # [128, T_total]
            )
        )
    return TK, Tb, T_total, cores


def build_program(TK, Tb, T_total, alpha, bsum, stage="full", repeat=1):
    import concourse.bacc as bacc
    import concourse.tile as tile
    from concourse import mybir
    from contextlib import ExitStack

    f32, bf16, i16 = mybir.dt.float32, mybir.dt.bfloat16, mybir.dt.int16
    AO = mybir.AluOpType

    nc = bacc.Bacc("TRN2", target_bir_lowering=False, debug=False,
                   num_devices=NC_CORES, num_swdge_queues=4)
    dbg_outs = {}
    if stage == "p1":
        dbg_outs["hdbg"] = nc.dram_tensor("hdbg", [SHP, 2 * N_H], bf16,
                                          kind="ExternalOutput")
    elif stage == "ag":
        dbg_outs["hfull_dbg"] = nc.dram_tensor(
            "hfull_dbg", [NC_CORES * SHP, 2 * N_H], bf16, kind="ExternalOutput")
    elif stage == "g1":
        for b in range(NBLK):
            dbg_outs[f"gdbg{b}"] = nc.dram_tensor(
                f"gdbg{b}", [128, int(Tb[b]), 2 * N_H], bf16,
                kind="ExternalOutput")

    xT_in = nc.dram_tensor("xT_in", [N_IN, 2 * SHP], bf16, kind="ExternalInput")
    W_in = nc.dram_tensor("w_in", [N_IN, N_H], bf16, kind="ExternalInput")
    iota_in = nc.dram_tensor("iota_in", [128, 128], f32, kind="ExternalInput")
    bias_in = nc.dram_tensor("bias_in", [128, 2], f32, kind="ExternalInput")
    wsum_in = nc.dram_tensor("wsum_in", [128, 1], bf16, kind="ExternalInput")
    dloc_in = nc.dram_tensor("dloc_in", [128, T_total], f32, kind="ExternalInput")
    wts_in = nc.dram_tensor("wts_in", [128, T_total], f32, kind="ExternalInput")
    idx_ins = [
        nc.dram_tensor(f"idx{b}_in", [128, int(Tb[b]) * 8], i16, kind="ExternalInput")
        for b in range(NBLK)
    ]
    z_out = nc.dram_tensor("z_out", [2, SHP], f32, kind="ExternalOutput")

    with tile.TileContext(nc) as tc, ExitStack() as ctx:
        dram = ctx.enter_context(tc.tile_pool(name="dram", bufs=1, space="DRAM"))
        h_shard = dram.tile([SHP, 2 * N_H], bf16)

        const = ctx.enter_context(tc.tile_pool(name="const", bufs=1))
        W_sb = const.tile([128, 2, N_H], bf16)
        nc.sync.dma_start(W_sb[:], W_in.ap().rearrange("(a p) h -> p a h", a=2))
        iota_sb = const.tile([128, 128], f32)
        nc.sync.dma_start(iota_sb[:], iota_in.ap())
        bias_sb = const.tile([128, 2], f32)
        nc.sync.dma_start(bias_sb[:], bias_in.ap())
        wsum_sb = const.tile([128, 1], bf16)
        nc.sync.dma_start(wsum_sb[:], wsum_in.ap())
        dloc_sb = const.tile([128, T_total], f32)
        nc.sync.dma_start(dloc_sb[:], dloc_in.ap())
        wts_sb = const.tile([128, T_total], f32)
        nc.sync.dma_start(wts_sb[:], wts_in.ap())

        for _rep in range(repeat):
            h_full = dram.tile([NC_CORES * SHP, 2 * N_H], bf16,
                               addr_space="Shared", tag=f"hf{_rep % 2}")
            # ---- phase 1: h = x @ W (both gcns) ----
            with tc.tile_pool(name="p1x", bufs=4) as p1x, \
                 tc.tile_pool(name="p1ps", bufs=2, space="PSUM") as p1ps, \
                 tc.tile_pool(name="p1h", bufs=3) as p1h:
                xg = xT_in.ap().rearrange("p (g n) -> p g n", g=2)
                for j in range(ND_TILES):
                    sl = slice(j * 128, (j + 1) * 128)
                    xa = p1x.tile([128, 2, 128], bf16, tag="xa")
                    nc.sync.dma_start(xa[:], xg[0:128, :, sl])
                    xb = p1x.tile([128, 2, 128], bf16, tag="xb")
                    nc.sync.dma_start(xb[:], xg[128:256, :, sl])
                    ph = p1ps.tile([128, 256], f32)
                    for g in range(2):
                        nc.tensor.matmul(ph[:, g * 128:(g + 1) * 128],
                                         lhsT=xa[:, g, :], rhs=W_sb[:, 0, :],
                                         start=True, stop=False)
                        nc.tensor.matmul(ph[:, g * 128:(g + 1) * 128],
                                         lhsT=xb[:, g, :], rhs=W_sb[:, 1, :],
                                         start=False, stop=True)
                    hs = p1h.tile([128, 256], bf16)
                    nc.vector.tensor_copy(hs[:], ph[:])
                    nc.sync.dma_start(h_shard[sl, :], hs[:])

            if stage == "p1":
                continue
            if stage == "p1t":
                nc.sync.dma_start(z_out.ap()[0:2, 0:128], dloc_sb[:, 0:2])
                continue

            if stage == "monly":
                # p1 + M-build only: no AG, no gather, no matmul
                with tc.tile_pool(name="mm", bufs=4) as mp, \
                     tc.tile_pool(name="acc", bufs=2) as accp:
                    for t_global in range(T_total):
                        M = mp.tile([128, 128], bf16)
                        nc.vector.tensor_scalar(
                            M[:], iota_sb[:],
                            dloc_sb[:, t_global:t_global + 1],
                            wts_sb[:, t_global:t_global + 1],
                            AO.is_equal, AO.mult)
                        if t_global % 64 == 63:
                            acc = accp.tile([128, 1], bf16, tag="acc")
                            nc.vector.tensor_reduce(
                                acc[:], M[:, 0:2], mybir.AxisListType.X, AO.max)
                nc.sync.dma_start(z_out.ap()[0:2, 0:128], dloc_sb[:, 0:2])
                continue

            # ---- phase 2: AllGather ----
            nc.gpsimd.collective_compute(
                "AllGather", AO.bypass,
                replica_groups=[list(range(NC_CORES))],
                ins=[h_shard[:]], outs=[h_full[:]],
            )

            if stage == "ag":
                continue
            if stage == "agt":
                nc.sync.dma_start(z_out.ap()[0:2, 0:128], dloc_sb[:, 0:2])
                continue

            if stage == "g1":
                with tc.tile_pool(name="msg", bufs=2) as msgp, \
                     tc.tile_pool(name="idx", bufs=4) as idxp:
                    for b in range(NBLK):
                        tb = 0
                        while tb < int(Tb[b]):
                            nchunk = min(CHUNK, int(Tb[b]) - tb)
                            it = idxp.tile([128, nchunk * 8], i16, tag=f"idx{b}")
                            nc.sync.dma_start(
                                it[:], idx_ins[b].ap()[:, tb * 8:(tb + nchunk) * 8])
                            mt = msgp.tile([128, nchunk, 2 * N_H], bf16,
                                           tag=f"msg{b}")
                            nc.gpsimd.dma_gather(
                                mt[:], h_full[b * BLK:(b + 1) * BLK, :], it[:],
                                nchunk * 128, nchunk * 128, 2 * N_H,
                                single_packet=False, queue_num=b % 4)
                            nc.sync.dma_start(
                                dbg_outs[f"gdbg{b}"].ap()[:, tb:tb + nchunk, :],
                                mt[:])
                            tb += nchunk
                continue

            if stage == "g4":
                # interleaved 4-queue gather: chunks round-robin across blocks
                with tc.tile_pool(name="msg", bufs=2) as msgp, \
                     tc.tile_pool(name="idx", bufs=4) as idxp, \
                     tc.tile_pool(name="acc", bufs=2) as accp:
                    tbs = [0] * NBLK
                    alive = True
                    while alive:
                        alive = False
                        for b in range(NBLK):
                            if tbs[b] >= int(Tb[b]):
                                continue
                            alive = True
                            tb = tbs[b]
                            nchunk = min(CHUNK, int(Tb[b]) - tb)
                            it = idxp.tile([128, nchunk * 8], i16, tag=f"idx{b}")
                            nc.sync.dma_start(
                                it[:], idx_ins[b].ap()[:, tb * 8:(tb + nchunk) * 8])
                            mt = msgp.tile([128, nchunk, 2 * N_H], bf16,
                                           tag=f"msg{b}")
                            nc.gpsimd.dma_gather(
                                mt[:], h_full[b * BLK:(b + 1) * BLK, :], it[:],
                                nchunk * 128, nchunk * 128, 2 * N_H,
                                single_packet=False, queue_num=b % 4)
                            acc = accp.tile([128, 1], f32, tag=f"acc{b}")
                            nc.vector.tensor_reduce(
                                acc[:], mt[:, 0, 0:2], mybir.AxisListType.X,
                                AO.max)
                            tbs[b] += nchunk
                continue

            if stage == "g3":
                # half-width gather (256B rows): desc-count identical to g2
                with tc.tile_pool(name="msg", bufs=2) as msgp, \
                     tc.tile_pool(name="idx", bufs=4) as idxp, \
                     tc.tile_pool(name="acc", bufs=2) as accp:
                    for b in range(NBLK):
                        tb = 0
                        while tb < int(Tb[b]):
                            nchunk = min(CHUNK, int(Tb[b]) - tb)
                            it = idxp.tile([128, nchunk * 8], i16, tag=f"idx{b}")
                            nc.sync.dma_start(
                                it[:], idx_ins[b].ap()[:, tb * 8:(tb + nchunk) * 8])
                            mt = msgp.tile([128, nchunk, N_H], bf16,
                                           tag=f"msg{b}")
                            nc.gpsimd.dma_gather(
                                mt[:], h_full[b * BLK:(b + 1) * BLK, 0:N_H],
                                it[:], nchunk * 128, nchunk * 128, N_H,
                                elem_step=2 * N_H,
                                single_packet=False, queue_num=b % 4)
                            acc = accp.tile([128, 1], f32, tag=f"acc{b}")
                            nc.vector.tensor_reduce(
                                acc[:], mt[:, 0, 0:2], mybir.AxisListType.X,
                                AO.max)
                            tb += nchunk
                continue

            if stage == "g2":
                with tc.tile_pool(name="msg", bufs=2) as msgp, \
                     tc.tile_pool(name="idx", bufs=4) as idxp, \
                     tc.tile_pool(name="acc", bufs=2) as accp:
                    for b in range(NBLK):
                        tb = 0
                        while tb < int(Tb[b]):
                            nchunk = min(CHUNK, int(Tb[b]) - tb)
                            it = idxp.tile([128, nchunk * 8], i16, tag=f"idx{b}")
                            nc.sync.dma_start(
                                it[:], idx_ins[b].ap()[:, tb * 8:(tb + nchunk) * 8])
                            mt = msgp.tile([128, nchunk, 2 * N_H], bf16,
                                           tag=f"msg{b}")
                            nc.gpsimd.dma_gather(
                                mt[:], h_full[b * BLK:(b + 1) * BLK, :], it[:],
                                nchunk * 128, nchunk * 128, 2 * N_H,
                                single_packet=False, queue_num=b % 4)
                            acc = accp.tile([128, 1], f32, tag=f"acc{b}")
                            nc.vector.tensor_reduce(
                                acc[:], mt[:, 0, 0:2], mybir.AxisListType.X,
                                AO.max)
                            tb += nchunk
                continue

            # ---- phase 3: gather + segment-sum + epilogue ----
            with tc.tile_pool(name="msg", bufs=2) as msgp, \
                 tc.tile_pool(name="idx", bufs=4) as idxp, \
                 tc.tile_pool(name="mm", bufs=4) as mp, \
                 tc.tile_pool(name="sps", bufs=2, space="PSUM") as sps, \
                 tc.tile_pool(name="hs2", bufs=3) as hs2p, \
                 tc.tile_pool(name="zps", bufs=2, space="PSUM") as zps, \
                 tc.tile_pool(name="ztmp", bufs=4) as ztp:
                tile_cnt = [0] * NBLK
                cur = [None] * NBLK
                t_global = 0
                for k in range(ND_TILES):
                    ps1 = sps.tile([128, 128], f32, tag="ps1")
                    ps2 = sps.tile([128, 128], f32, tag="ps2")
                    tiles_k = [(b, i) for b in range(NBLK)
                               for i in range(int(TK[k, b]))]
                    for ti, (b, _) in enumerate(tiles_k):
                        tb = tile_cnt[b]
                        if cur[b] is None or tb - cur[b][1] >= CHUNK:
                            nchunk = min(CHUNK, int(Tb[b]) - tb)
                            mt = msgp.tile([128, nchunk, 2 * N_H], bf16,
                                           tag=f"msg{b}")
                            if stage != "nogather":
                                it = idxp.tile([128, nchunk * 8], i16,
                                               tag=f"idx{b}")
                                nc.sync.dma_start(
                                    it[:],
                                    idx_ins[b].ap()[:, tb * 8:(tb + nchunk) * 8])
                                nc.gpsimd.dma_gather(
                                    mt[:], h_full[b * BLK:(b + 1) * BLK, :],
                                    it[:], nchunk * 128, nchunk * 128, 2 * N_H,
                                    single_packet=False, queue_num=b % 4)
                            cur[b] = (mt, tb)
                        mt, base = cur[b]
                        slot = tb - base
                        M = mp.tile([128, 128], bf16)
                        nc.vector.tensor_scalar(
                            M[:], iota_sb[:],
                            dloc_sb[:, t_global:t_global + 1],
                            wts_sb[:, t_global:t_global + 1],
                            AO.is_equal, AO.mult)
                        first = ti == 0
                        last = ti == len(tiles_k) - 1
                        nc.tensor.matmul(ps1[:], lhsT=mt[:, slot, 0:N_H],
                                         rhs=M[:], start=first, stop=last)
                        nc.tensor.matmul(ps2[:], lhsT=mt[:, slot, N_H:2 * N_H],
                                         rhs=M[:], start=first, stop=last)
                        tile_cnt[b] += 1
                        t_global += 1

                    hs = hs2p.tile([128, 512], bf16)
                    ACT = mybir.ActivationFunctionType.Relu
                    nc.scalar.activation(hs[:, 0:128], ps1[:], ACT,
                                         bias=bias_sb[:, 0:1], scale=1.0)
                    nc.scalar.activation(hs[:, 128:256], ps1[:], ACT,
                                         bias=bias_sb[:, 1:2], scale=-1.0)
                    nc.scalar.activation(hs[:, 256:384], ps2[:], ACT,
                                         bias=bias_sb[:, 0:1], scale=1.0)
                    nc.scalar.activation(hs[:, 384:512], ps2[:], ACT,
                                         bias=bias_sb[:, 1:2], scale=-1.0)
                    zp = zps.tile([1, 512], f32)
                    nc.tensor.matmul(zp[:], lhsT=wsum_sb[:], rhs=hs[:],
                                     start=True, stop=True)
                    wk = SH - k * 128 if (k + 1) * 128 > SH else 128
                    c0 = k * 128
                    tz1 = ztp.tile([1, 128], f32, tag="tz1")
                    nc.vector.tensor_scalar(tz1[:, :wk], zp[0:1, 128:128 + wk],
                                            -alpha, bsum, AO.mult, AO.add)
                    zf1 = ztp.tile([1, 128], f32, tag="zf1")
                    nc.vector.tensor_tensor(zf1[:, :wk], zp[0:1, 0:wk],
                                            tz1[:, :wk], op=AO.add)
                    nc.sync.dma_start(z_out.ap()[0:1, c0:c0 + wk], zf1[:, :wk])
                    tz2 = ztp.tile([1, 128], f32, tag="tz2")
                    nc.vector.tensor_scalar(tz2[:, :wk], zp[0:1, 384:384 + wk],
                                            -alpha, bsum, AO.mult, AO.add)
                    zf2 = ztp.tile([1, 128], f32, tag="zf2")
                    nc.vector.tensor_tensor(zf2[:, :wk], zp[0:1, 256:256 + wk],
                                            tz2[:, :wk], op=AO.add)
                    nc.sync.dma_start(z_out.ap()[1:2, c0:c0 + wk], zf2[:, :wk])

        if stage == "p1":
            nc.sync.dma_start(dbg_outs["hdbg"].ap(), h_shard[:])
        elif stage == "ag":
            nc.sync.dma_start(dbg_outs["hfull_dbg"].ap(), h_full[:])

    nc.compile()
    return nc


def make_in_maps(x_1, x_2, W_gcn, gcn_bias, lin_W, cores, T_total):
    x1 = np.asarray(x_1)[0]
    x2 = np.asarray(x_2)[0]
    W_bf = np.asarray(W_gcn, dtype=np.float32).astype(BF16)
    iota = np.tile(np.arange(128, dtype=np.float32), (128, 1))
    bias2 = np.stack([np.asarray(gcn_bias, np.float32),
                      -np.asarray(gcn_bias, np.float32)], axis=1)
    wsum = np.asarray(lin_W, np.float32).sum(axis=1, keepdims=True).astype(BF16)
    in_maps = []
    for c in range(NC_CORES):
        sl = slice(c * SH, (c + 1) * SH)
        xT = np.zeros((N_IN, 2 * SHP), dtype=BF16)
        xT[:, :SH] = x1[sl].T.astype(BF16)
        xT[:, SHP:SHP + SH] = x2[sl].T.astype(BF16)
        m = {
            "xT_in": xT,
            "w_in": W_bf,
            "iota_in": iota,
            "bias_in": np.ascontiguousarray(bias2),
            "wsum_in": wsum,
            "dloc_in": cores[c]["dloc"],
            "wts_in": cores[c]["wts"],
        }
        for b in range(NBLK):
            m[f"idx{b}_in"] = cores[c]["idx"][b]
        in_maps.append(m)
    return in_maps


def kernel(x_1, x_2, edge_index, edge_weight, W_gcn, gcn_bias, prelu_a,
           lin_W, lin_b):
    from concourse.bass_utils import run_bass_kernel_spmd

    TK, Tb, T_total, cores = prepare(edge_index, edge_weight)
    alpha = float(np.asarray(prelu_a).reshape(-1)[0])
    bsum = float(np.asarray(lin_b, dtype=np.float32).sum())
    nc = build_program(TK, Tb, T_total, alpha, bsum)
    in_maps = make_in_maps(x_1, x_2, W_gcn, gcn_bias, lin_W, cores, T_total)
    res = run_bass_kernel_spmd(nc, in_maps, core_ids=list(range(NC_CORES)))
    z1 = np.concatenate([res.results[c]["z_out"][0, :SH] for c in range(NC_CORES)])
    z2 = np.concatenate([res.results[c]["z_out"][1, :SH] for c in range(NC_CORES)])
    return np.concatenate([z1, z2]).astype(np.float32)

